# revision 1
# baseline (speedup 1.0000x reference)
"""Causal self-attention head (softmax over the QUERY axis) on 8 trn2 cores.

Reference math (note the unusual softmax axis=-2, i.e. per key-column):
    q = x @ Wq; k = x @ Wk; v = x @ Wv            # [B,T,64]
    s[b,q,k] = (q . k) * 64**-0.5, masked to q >= k
    w[:, k]  = softmax over q of s[:, k]           # column softmax
    out[b,q,:] = sum_k w[q,k] v[k,:]

Because the softmax normalizes over q (the contraction axis of the second
matmul is k), the normalizer folds into a per-key scaling of v:
    out[q] = sum_{k<=q} exp(s[q,k]) * (r[k] * v[k]),  r[k] = 1/sum_{q>=k} exp(s[q,k])

Sharding: 8 cores = 4 batches x 2 "parities". Core (b, p) owns the key-column
blocks kb = 2i+p (i=0..15, 128 columns each) of batch b and produces a partial
output over all q; the host adds the two parity partials per batch.

To keep the SPMD program identical on all cores, parity-1 cores receive x^T
shifted left by 128 columns (zero-padded tail); their key blocks then sit at
the same compile-time offsets as parity-0's, and the host shifts their output
back by +128 q positions. Garbage q-columns from the zero pad are killed by a
per-core "tailmask" input.

Pipeline: x^T chunks are processed in DESCENDING order, so each chunk's two
key blocks can run their scores+exp immediately (they only need q^T columns
from chunks already loaded). The ACT engine's exp work therefore overlaps the
projection matmuls from the start, keeping the PE dense (HAM clock-gate warm).
The output chunks run as a dense PE tail.

Other notes:
- The chunk's two key blocks are projected with ONE matmul per contraction
  subtile using a strided 3D moving-operand access pattern (N=256); k and v
  share one psum bank.
- Causal masks are applied by accumulating triangular-count matmuls into the
  scores PSUM (PE-local).
- exp runs on ACT over [128,1024] psum tiles with the 1/sqrt(64) folded into
  its free affine pre-scale and per-column sums from fused accum_out.
- v^T -> v-natural transposes run on the DMA xbar (bf16), costing no PE/PSUM.
- PSUM->SBUF copies run on the vector engine (ACT is the exp bottleneck).
"""

import os
import sys

import numpy as np

for _p in ("/opt/trn_rl_repo",):
    if _p not in sys.path:
        sys.path.insert(0, _p)

import concourse.bass as bass
import concourse.mybir as mybir
from concourse import bacc
from concourse.bass_utils import run_bass_kernel_spmd
from concourse.tile import TileContext

B, T, CE, CH = 4, 4096, 1024, 64
P = 128
NB = 16          # key blocks per core (128 cols each)
NCHUNK = 8       # 512-col chunks covering T
SCALE = CH ** -0.5
NEG = -1e30
M0 = NEG / P     # per-unit mask magnitude for the triangular-count mask
ETILE = 1024     # scores/exp tile width (2 psum banks)

F32 = mybir.dt.float32
BF16 = mybir.dt.bfloat16

N_CORES = 8

# Results of the last run (for test harnesses: exec_time_ns etc.)
LAST_RESULTS = None


def _build_program():
    # Bacc (not plain Bass): its compile() pipeline legalizes multi-semaphore
    # waits into EventSemaphore instructions and moves matmul waits onto
    # LDWEIGHTS — required by the 1-wait-per-instruction hardware encoding.
    nc = bacc.Bacc("TRN2", target_bir_lowering=False, debug=False)

    xT = nc.declare_dram_parameter("xT", [CE, T], BF16, isOutput=False)
    wq = nc.declare_dram_parameter("wq", [CE, CH], BF16, isOutput=False)
    wk = nc.declare_dram_parameter("wk", [CE, CH], BF16, isOutput=False)
    wv = nc.declare_dram_parameter("wv", [CE, CH], BF16, isOutput=False)
    tailmask = nc.declare_dram_parameter("tailmask", [P, P], BF16, isOutput=False)
    outT = nc.declare_dram_parameter("outT", [CH, T], F32, isOutput=True)

    with TileContext(nc) as tc:
        with (
            tc.tile_pool(name="consts", bufs=1) as consts,
            tc.tile_pool(name="qkv", bufs=1) as qkv,
            tc.tile_pool(name="w2p", bufs=1) as w2p,
            tc.tile_pool(name="xp", bufs=8) as xp,
            tc.tile_pool(name="pp", bufs=2, space="PSUM") as pp,
            tc.tile_pool(name="sp", bufs=2, space="PSUM") as sp,
        ):
            # ---- DMA'd constants ----
            wq_sb = consts.tile([P, CE // P, CH], BF16, tag="wq")
            wk_sb = consts.tile([P, CE // P, CH], BF16, tag="wk")
            wv_sb = consts.tile([P, CE // P, CH], BF16, tag="wv")
            nc.sync.dma_start(wq_sb[:], wq.rearrange("(o p) f -> p o f", p=P))
            nc.sync.dma_start(wk_sb[:], wk.rearrange("(o p) f -> p o f", p=P))
            nc.sync.dma_start(wv_sb[:], wv.rearrange("(o p) f -> p o f", p=P))
            tmask = consts.tile([P, P], BF16, tag="tmask")
            nc.sync.dma_start(tmask[:], tailmask[:])

            # ---- gpsimd-built mask constants ----
            # Atri[ch, p] = 1 if ch < p else 0; Bneg[ch, c] = M0 if c <= ch
            # => (Atri^T @ Bneg)[p, c] = M0 * max(0, p - c): the causal mask.
            ones = consts.tile([P, P], BF16, tag="ones")
            nc.gpsimd.memset(ones[:], 1.0)
            atri = consts.tile([P, P], BF16, tag="atri")
            nc.gpsimd.memset(atri[:], 1.0)
            nc.gpsimd.affine_select(
                out=atri[:],
                in_=atri[:],
                compare_op=mybir.AluOpType.is_ge,
                fill=0.0,
                base=-1,
                pattern=[[1, P]],
                channel_multiplier=-1,
            )
            bneg = consts.tile([P, 2 * P], BF16, tag="bneg")
            nc.gpsimd.memset(bneg[:], M0)
            nc.gpsimd.affine_select(
                out=bneg[:],
                in_=bneg[:],
                compare_op=mybir.AluOpType.is_ge,
                fill=0.0,
                base=0,
                pattern=[[-1, 2 * P]],
                channel_multiplier=1,
            )

            # ---- persistent activations ----
            qT = qkv.tile([CH, T], BF16, tag="qT")
            kTl = qkv.tile([CH, NB * P], BF16, tag="kTl")
            vT = qkv.tile([CH, NB * P], BF16, tag="vT")
            vnat = qkv.tile([P, NB, CH], BF16, tag="vnat")
            stats = qkv.tile([P, NB, 4], F32, tag="stats")
            ssum = qkv.tile([P, NB], F32, tag="ssum")
            rr = qkv.tile([P, NB], F32, tag="rr")
            outsb = qkv.tile([CH, T], F32, tag="outsb")

            w2 = [
                w2p.tile([P, T - 256 * i], BF16, tag=f"w2_{i}", name=f"w2_{i}")
                for i in range(NB)
            ]

            # PE warm-up spam: keeps the HAM clock-gate open while the first
            # input DMAs land (also absorbs the gpsimd-consts wait).
            for t in range(40):
                dscr = sp.tile([CH, 512], F32, tag="po", name=f"warm{t}")
                nc.tensor.matmul(
                    dscr[:, 0:1], ones[0:CH, 0:CH], ones[0:CH, 0:1],
                    start=True, stop=True,
                )
            dscr = sp.tile([CH, 512], F32, tag="po", name="abs_tm")
            nc.tensor.matmul(
                dscr[0:1, 0:1], tmask[0:CH, 0:1], tmask[0:CH, 0:1],
                start=True, stop=True,
            )

            def emit_block(i):
                lhs = kTl[:, P * i : P * (i + 1)]
                qlo = 256 * i
                L = T - qlo
                net = (L + ETILE - 1) // ETILE
                for t3 in range(net):
                    w3 = min(ETILE, L - ETILE * t3)
                    last3 = t3 == net - 1
                    sc = sp.tile([P, ETILE], F32, tag="sc")
                    nsub = (w3 + 511) // 512
                    for u in range(nsub):
                        wu = min(512, w3 - 512 * u)
                        qs = qlo + ETILE * t3 + 512 * u
                        # each 512-col sub-mm fills its own PSUM bank:
                        # start=True per bank (start clears only the
                        # addressed bank's has_written bits)
                        nc.tensor.matmul(
                            sc[:, 512 * u : 512 * u + wu],
                            lhs,
                            qT[:, qs : qs + wu],
                            start=True,
                            stop=(u == nsub - 1 and t3 != 0 and not last3),
                            skip_group_check=True,
                        )
                    if t3 == 0:
                        # causal mask: += M0 * max(0, p - col)
                        nc.tensor.matmul(
                            sc[:, 0:256],
                            atri[:],
                            bneg[:],
                            start=False,
                            stop=not last3,
                        )
                    if last3:
                        # zero-pad tail kill on the final 128 columns
                        nc.tensor.matmul(
                            sc[:, w3 - P : w3],
                            ones[:],
                            tmask[:],
                            start=False,
                            stop=True,
                        )
                    nc.scalar.activation(
                        w2[i][:, ETILE * t3 : ETILE * t3 + w3],
                        sc[:, :w3],
                        mybir.ActivationFunctionType.Exp,
                        scale=SCALE,
                        accum_out=stats[:, i, t3 : t3 + 1],
                    )
                nc.vector.reduce_sum(
                    ssum[:, i : i + 1],
                    stats[:, i, 0:net],
                    axis=mybir.AxisListType.X,
                )
                nc.vector.reciprocal(rr[:, i : i + 1], ssum[:, i : i + 1])
                nc.vector.tensor_scalar_mul(
                    vnat[:, i, :], vnat[:, i, :], rr[:, i : i + 1]
                )

            # ======== merged pipeline: chunks descending, scores inline ========
            def process_chunk(j):
                xtile = xp.tile([P, CE // P, 512], BF16, tag="xtile")
                dma_eng = nc.sync if j % 2 == 0 else nc.scalar
                dma_eng.dma_start(
                    xtile[:],
                    xT[:, 512 * j : 512 * (j + 1)].rearrange(
                        "(o p) f -> p o f", p=P
                    ),
                )
                # absorber: put this chunk's DMA wait on a throwaway MM
                dscr = sp.tile([CH, 512], F32, tag="po", name=f"absx{j}")
                nc.tensor.matmul(
                    dscr[0:1, 0:1],
                    xtile[:, 0, 0:1],
                    xtile[:, 0, 0:1],
                    start=True,
                    stop=True,
                )

                # q projection: full 512 columns
                psq = pp.tile([CH, 512], F32, tag="proj")
                for s in range(CE // P):
                    nc.tensor.matmul(
                        psq[:],
                        wq_sb[:, s, :],
                        xtile[:, s, :],
                        start=(s == 0),
                        stop=(s == CE // P - 1),
                    )
                nc.vector.tensor_copy(qT[:, 512 * j : 512 * (j + 1)], psq[:])

                # k/v: the chunk's two key blocks (offsets 0 and 256) in ONE
                # matmul per subtile via a strided moving operand; k and v
                # share one psum tile (k in [0:256], v in [256:512])
                pskv = pp.tile([CH, 512], F32, tag="proj")
                for s in range(CE // P):
                    kvrhs = xtile[:, s, :].rearrange(
                        "p (b c) -> p b c", c=P
                    )[:, 0::2, :]
                    nc.tensor.matmul(
                        pskv[:, 0:256].rearrange("p (b c) -> p b c", c=P),
                        wk_sb[:, s, :],
                        kvrhs,
                        start=(s == 0),
                        stop=False,
                        skip_group_check=True,
                    )
                for s in range(CE // P):
                    kvrhs = xtile[:, s, :].rearrange(
                        "p (b c) -> p b c", c=P
                    )[:, 0::2, :]
                    nc.tensor.matmul(
                        pskv[:, 256:512].rearrange("p (b c) -> p b c", c=P),
                        wv_sb[:, s, :],
                        kvrhs,
                        start=False,
                        stop=(s == CE // P - 1),
                        skip_group_check=True,
                    )
                nc.vector.tensor_copy(kTl[:, 256 * j : 256 * (j + 1)], pskv[:, 0:256])
                nc.vector.tensor_copy(vT[:, 256 * j : 256 * (j + 1)], pskv[:, 256:512])

                # v -> natural layout via DMA xbar transpose (bf16, no PE)
                for half in (0, 1):
                    i = 2 * j + half
                    nc.sync.dma_start_transpose(
                        vnat[:, i, :], vT[:, P * i : P * (i + 1)]
                    )

                emit_block(2 * j)
                emit_block(2 * j + 1)

            for j in reversed(range(NCHUNK)):
                process_chunk(j)


            # ============ output tail: out^T = sum_i vr_i^T @ w2_i ============
            for c in range(NCHUNK):
                po = sp.tile([CH, 512], F32, tag="po", name=f"po{c}")
                ilast = min(2 * c + 1, NB - 1)
                for i in range(ilast + 1):
                    off = 512 * c - 256 * i
                    if off >= 0:
                        nc.tensor.matmul(
                            po[:],
                            vnat[:, i, :],
                            w2[i][:, off : off + 512],
                            start=(i == 0),
                            stop=(i == ilast),
                        )
                    else:
                        nc.tensor.matmul(
                            po[:, 256:512],
                            vnat[:, i, :],
                            w2[i][:, 0:256],
                            start=False,
                            stop=(i == ilast),
                        )
                nc.vector.tensor_copy(outsb[:, 512 * c : 512 * (c + 1)], po[:])
                nc.sync.dma_start(
                    outT[:, 512 * c : 512 * (c + 1)],
                    outsb[:, 512 * c : 512 * (c + 1)],
                )

    return nc


_PROGRAM = None


def _get_program():
    global _PROGRAM
    if _PROGRAM is None:
        nc = _build_program()
        nc.finalize()
        _PROGRAM = nc
    return _PROGRAM


def kernel(x, Wk, Wq, Wv, trace=False, trace_cores=None):
    global LAST_RESULTS
    x = np.asarray(x)
    Wk = np.asarray(Wk)
    Wq = np.asarray(Wq)
    Wv = np.asarray(Wv)

    import ml_dtypes

    bf = ml_dtypes.bfloat16
    wq_b = Wq.astype(bf)
    wk_b = Wk.astype(bf)
    wv_b = Wv.astype(bf)

    zeros_mask = np.zeros((P, P), bf)
    neg_mask = np.full((P, P), NEG / P, bf)

    in_maps = []
    for c in range(N_CORES):
        b, parity = c // 2, c % 2
        xTb = np.ascontiguousarray(x[b].T).astype(bf)  # [CE, T]
        if parity:
            xTb = np.concatenate([xTb[:, P:], np.zeros((CE, P), bf)], axis=1)
        in_maps.append(
            {
                "xT": np.ascontiguousarray(xTb),
                "wq": wq_b,
                "wk": wk_b,
                "wv": wv_b,
                "tailmask": neg_mask if parity else zeros_mask,
            }
        )

    nc = _get_program()
    res = run_bass_kernel_spmd(
        nc,
        in_maps,
        list(range(N_CORES)),
        trace=trace,
        **({"trace_cores": trace_cores} if trace_cores is not None else {}),
    )
    LAST_RESULTS = res

    out = np.zeros((B, T, CH), np.float32)
    for c in range(N_CORES):
        b, parity = c // 2, c % 2
        oT = np.asarray(res.results[c]["outT"], np.float32)  # [CH, T]
        if parity:
            # core's column t corresponds to q = t + 128
            out[b, P:, :] += oT[:, : T - P].T
        else:
            out[b] += oT.T
    return out

